# revision 19
# baseline (speedup 1.0000x reference)
"""GATv2 layer on 8 Trainium2 NeuronCores (Bass/Tile), v2.

Strategy (edge-parallel by target-node slice, no collectives, no scatters):
  - Node n belongs to core n // 12500. Targets grouped into 98 aligned blocks
    of 128 nodes; cells = (chunk r of 25088 src nodes, block b). Edges sorted
    (r, b), padded per-cell to 128-slot tiles (tile counts maxed over cores so
    all 8 cores run one SPMD program).
  - h_l for the core's slice and a [128, 98, 68] accumulator live in SBUF for
    the whole run -> no h_l gather, no scatter_add, no HBM merge phase.
  - h_r chunk tables ([25088, 64] f32, lane-permuted rows) are built on the
    fly; chunk r+1's table build is interleaved into chunk r's edge spans so
    PE/DMA never serialize against the gather stream.
  - Per (r, span of 8 blocks): one dma_gather fetches x_j rows (1 descriptor
    per slot, 256B, 4-queue rotation ~ 1.6ns/desc + B/137GB/s measured).
    x_i is expanded on-chip from SBUF h_l with one-hot selT matmuls (bf16);
    aggregation uses one-hot selq matmuls accumulating per-cell in PSUM, then
    a tiny DVE add into the SBUF accumulator.
  - Final: per block normalize (divide by exp-sum) + bias, DMA out.

Numerics: softmax without max-shift (scores O(+-12), safe in f32); messages,
sel matrices and h_l in bf16; scores in f32. rel err ~1e-3.
"""

import sys
import types

sys.path.insert(0, "/opt/trn_rl_repo")

import numpy as np

N, E, F_IN, H, F_OUT = 100000, 1600000, 128, 4, 16
HF = H * F_OUT            # 64
NEG_SLOPE = 0.2
NCORES = 8
NLOC = N // NCORES        # 12500
NLOCP = 12544             # 98*128
NB = NLOCP // 128         # 98 target blocks per core
CHUNK = 25088             # 196*128 src rows per chunk table (int16-safe rows)
NCHUNK = 4
NPAD = NCHUNK * CHUNK     # 100352 padded feature columns
SPB = 5                   # target blocks per span (gather batch)
NSPAN = -(-NB // SPB)     # 13
SB = 8                    # tiles per expansion sub-batch (psum 8*64 f32)


# ----------------------------------------------------------------- host prep
def _wrap16(ix):
    """int16 index layout for dma_gather: i -> (i%16, i//16), x8."""
    w2 = ix.reshape(-1, 16).T
    return np.tile(w2, (8, 1)).copy()


def prep(edge_index):
    src = np.asarray(edge_index[0], dtype=np.int64).astype(np.int32)
    tgt = np.asarray(edge_index[1], dtype=np.int64).astype(np.int32)
    percore = []
    cnts = np.zeros((NCORES, NCHUNK * NB), np.int64)
    for c in range(NCORES):
        n0 = c * NLOC
        m = (tgt >= n0) & (tgt < n0 + NLOC)
        s, t = src[m], tgt[m] - n0
        b = t >> 7
        r = s // CHUNK
        sloc = s - r * CHUNK
        tile = sloc >> 7
        # table row layout: store unit u holds tiles 4u..4u+3 lane-interleaved
        row = 512 * (tile >> 2) + 4 * (sloc & 127) + (tile & 3)
        cell = r * NB + b
        order = np.argsort(cell, kind="stable")
        cnts[c] = np.bincount(cell, minlength=NCHUNK * NB)
        percore.append((row[order].astype(np.int32),
                        (t & 127)[order].astype(np.int8), cnts[c]))
    T = (-(-cnts.max(axis=0) // 128)).astype(np.int64)   # tiles per cell
    cello = np.concatenate([[0], np.cumsum(T * 128)])
    S = int(cello[-1])
    outs = []
    for c in range(NCORES):
        row, tq, cellcnt = percore[c]
        xi = np.zeros(S, np.int16)
        ts = np.full(S, -1, np.int8)
        eo = np.concatenate([[0], np.cumsum(cellcnt)])
        cl = np.repeat(np.arange(NCHUNK * NB), cellcnt)
        pos = cello[cl] + (np.arange(len(row)) - eo[cl])
        xi[pos] = row
        ts[pos] = tq
        outs.append({
            "xj_idx": _wrap16(xi),
            "tsel_rep": np.ascontiguousarray(
                np.broadcast_to(ts.astype(np.int16)[None, :], (128, S))),
            "tsel_col": np.ascontiguousarray(ts.reshape(-1, 128).T),
        })
    return tuple(T.tolist()), outs


# ------------------------------------------------------------- device kernel
def build(Tkey):
    import concourse.mybir as mybir
    import concourse.tile as tile
    from concourse import bacc

    dt = mybir.dt
    AF = mybir.ActivationFunctionType
    OP = mybir.AluOpType

    Tarr = np.asarray(Tkey, np.int64)                    # [NCHUNK*NB] r-major
    tile_off = np.concatenate([[0], np.cumsum(Tarr)])
    S = int(Tarr.sum()) * 128
    spans = []   # (r, t0, T_rg, cells=[(b, tcnt), ...])
    for r in range(NCHUNK):
        for g in range(NSPAN):
            b0, b1 = g * SPB, min((g + 1) * SPB, NB)
            cells = [(b, int(Tarr[r * NB + b])) for b in range(b0, b1)
                     if Tarr[r * NB + b] > 0]
            T_rg = sum(t for _, t in cells)
            if T_rg:
                spans.append((r, int(tile_off[r * NB + b0]), T_rg, cells))
    TM = max(sp[2] for sp in spans)

    nc = bacc.Bacc("TRN2", target_bir_lowering=False, num_swdge_queues=4,
                   dynamic_dma_scratch_size=57344)

    featT = nc.dram_tensor("featT", [128, NPAD], dt.float32,
                           kind="ExternalInput")
    featT_loc = nc.dram_tensor("featT_loc", [128, NLOCP], dt.float32,
                               kind="ExternalInput")
    w_lr = nc.dram_tensor("w_lr", [128, 2 * HF], dt.float32,
                          kind="ExternalInput")     # [:,:64]=w_l [:,64:]=w_r
    att_b = nc.dram_tensor("att_b", [128, HF], dt.float32,
                           kind="ExternalInput")
    bias_b = nc.dram_tensor("bias_b", [128, HF], dt.float32,
                            kind="ExternalInput")
    xj_idx = nc.dram_tensor("xj_idx", [128, S // 16], dt.int16,
                            kind="ExternalInput")
    tsel_rep = nc.dram_tensor("tsel_rep", [128, S], dt.int16,
                              kind="ExternalInput")
    tsel_col = nc.dram_tensor("tsel_col", [128, S // 128], dt.int8,
                              kind="ExternalInput")
    hr = [nc.dram_tensor(f"hr{r}", [CHUNK, HF], dt.float32, kind="Internal")
          for r in range(NCHUNK)]
    out = nc.dram_tensor("out", [NLOCP, HF], dt.float32, kind="ExternalOutput")

    qi = 0
    from contextlib import ExitStack
    with tile.TileContext(nc) as tc:
        with ExitStack() as es:
            P = lambda *a, **k: es.enter_context(tc.tile_pool(*a, **k))
            cst = P(name="cst", bufs=1)
            hlp = P(name="hlp", bufs=1)
            accp = P(name="accp", bufs=1)
            hfp = P(name="hfp", bufs=2)
            hsp = P(name="hsp", bufs=2)
            hpp = P(name="hpp", bufs=2, space="PSUM")
            ixp = P(name="ixp", bufs=2)
            trp = P(name="trp", bufs=2)
            tcp = P(name="tcp", bufs=2)
            xjp = P(name="xjp", bufs=4)
            seltp = P(name="seltp", bufs=2)
            selqp = P(name="selqp", bufs=4)
            zpa = P(name="zpa", bufs=2)
            zpb = P(name="zpb", bufs=1)
            mp = P(name="mp", bufs=3)
            scp = P(name="scp", bufs=3)
            pz = P(name="pz", bufs=3, space="PSUM")
            pw = P(name="pw", bufs=3, space="PSUM")
            outp = P(name="outp", bufs=2)
            # ---------------- constants ----------------------------------
            wt = cst.tile([128, 2 * HF], dt.float32)
            nc.sync.dma_start(out=wt[:], in_=w_lr[:])
            attf = cst.tile([128, HF], dt.float32)
            nc.sync.dma_start(out=attf[:], in_=att_b[:])
            att_t = cst.tile([128, HF], dt.bfloat16)
            nc.vector.tensor_copy(out=att_t[:], in_=attf[:])
            bias_t = cst.tile([128, HF], dt.float32)
            nc.sync.dma_start(out=bias_t[:], in_=bias_b[:])
            iota_i = cst.tile([128, 128], dt.int32)
            nc.gpsimd.iota(iota_i[:], pattern=[[1, 128]], base=0,
                           channel_multiplier=0)
            iota_row = cst.tile([128, 128], dt.int8)
            nc.vector.tensor_copy(out=iota_row[:], in_=iota_i[:])
            iota_c = cst.tile([128, 1], dt.int32)
            nc.gpsimd.iota(iota_c[:], pattern=[[0, 1]], base=0,
                           channel_multiplier=1)
            iota_col = cst.tile([128, 1], dt.float32)
            nc.vector.tensor_copy(out=iota_col[:], in_=iota_c[:])
            iota_col8 = cst.tile([128, 1], dt.int8)
            nc.vector.tensor_copy(out=iota_col8[:], in_=iota_c[:])
            ident = cst.tile([128, 128], dt.float32)
            nc.vector.tensor_tensor(
                out=ident[:], in0=iota_row[:],
                in1=iota_col8[:].to_broadcast([128, 128]), op=OP.is_equal)

            ic16 = cst.tile([128, 1], dt.int16)
            nc.vector.tensor_copy(out=ic16[:], in_=iota_c[:])
            partcol16 = cst.tile([128, TM * 128], dt.int16)
            nc.vector.tensor_copy(
                out=partcol16[:],
                in_=ic16[:].to_broadcast([128, TM * 128]))
            acc_t = accp.tile([128, NB * (HF + H)], dt.float32)
            nc.vector.memset(acc_t[:], 0.0)
            h_l_sb = hlp.tile([128, NB * HF], dt.bfloat16)

            # ---------------- phase H units ------------------------------
            def h_unit_table(r, u):
                ft = hfp.tile([128, 512], dt.float32, tag="ft")
                nc.scalar.dma_start(
                    out=ft[:],
                    in_=featT[:, r * CHUNK + 512 * u:r * CHUNK + 512 * (u + 1)])
                hp = hpp.tile([128, 256], dt.float32, space="PSUM")
                for j in range(4):
                    nc.tensor.matmul(out=hp[:, 64 * j:64 * (j + 1)],
                                     lhsT=ft[:, 128 * j:128 * (j + 1)],
                                     rhs=wt[:, HF:], start=True, stop=True)
                hs = hsp.tile([128, 256], dt.float32, tag="hs")
                nc.vector.tensor_copy(out=hs[:], in_=hp[:])
                nc.scalar.dma_start(out=hr[r][512 * u:512 * (u + 1), :],
                                    in_=hs[:])

            def h_unit_hl(v):
                c0 = 512 * v
                c1 = min(512 * (v + 1), NLOCP)
                w, ntl = c1 - c0, (c1 - c0) // 128
                fl = hfp.tile([128, 512], dt.float32, tag="ft")
                nc.scalar.dma_start(out=fl[:, :w], in_=featT_loc[:, c0:c1])
                hp = hpp.tile([128, 256], dt.float32, space="PSUM")
                for j in range(ntl):
                    nc.tensor.matmul(out=hp[:, 64 * j:64 * (j + 1)],
                                     lhsT=fl[:, 128 * j:128 * (j + 1)],
                                     rhs=wt[:, :HF], start=True, stop=True)
                nc.vector.tensor_copy(out=h_l_sb[:, c0 // 2:c0 // 2 + ntl * 64],
                                      in_=hp[:, :ntl * 64])

            for u in range(CHUNK // 512):         # table 0 first
                h_unit_table(0, u)
            for v in range(-(-NLOCP // 512)):     # h_l
                h_unit_hl(v)

            # ---------------- edge spans ---------------------------------
            # 4-stage software pipeline over spans: every op's producer ran a
            # full iteration earlier, so no engine queue blocks at its head.
            Nsp = len(spans)
            st = {}

            def loads(i):
                r, t0, T_rg, cells = spans[i]
                n = 128 * T_rg
                ix = ixp.tile([128, TM * 8], dt.int16, tag="ix")
                nc.sync.dma_start(out=ix[:, :8 * T_rg],
                                  in_=xj_idx[:, 8 * t0:8 * (t0 + T_rg)])
                tr = trp.tile([128, TM * 128], dt.int16, tag="tr")
                nc.sync.dma_start(out=tr[:, :n],
                                  in_=tsel_rep[:, 128 * t0:128 * t0 + n])
                tcs = tcp.tile([128, TM], dt.int8, tag="tc")
                nc.sync.dma_start(out=tcs[:, :T_rg],
                                  in_=tsel_col[:, t0:t0 + T_rg])
                st[i] = {"ix": ix, "tr": tr, "tc": tcs}

            def gather(i):
                nonlocal qi
                r, t0, T_rg, cells = spans[i]
                xj = xjp.tile([128, TM * HF], dt.float32, tag="xj")
                # split into two half-gathers on distinct queues: each half's
                # descriptor set fits the SWDGE ring, so Q7 generation never
                # flow-controls on drain and the two halves drain in parallel.
                th = (T_rg + 1) // 2
                for a, b in ((0, th), (th, T_rg)):
                    if b <= a:
                        continue
                    n = 128 * (b - a)
                    nc.gpsimd.dma_gather(
                        xj[:, a * HF:b * HF].rearrange("p (t f) -> p t f",
                                                       f=HF),
                        hr[r][:], st[i]["ix"][:, 8 * a:8 * b], n, n, HF,
                        single_packet=False, queue_num=qi % 4)
                    qi += 1
                st[i]["xj"] = xj

            def sels(i):
                r, t0, T_rg, cells = spans[i]
                n = 128 * T_rg
                selT = seltp.tile([128, TM * 128], dt.bfloat16, tag="selT")
                nc.vector.tensor_tensor(
                    out=selT[:, :n], in0=st[i]["tr"][:, :n],
                    in1=partcol16[:, :n], op=OP.is_equal)
                selq = selqp.tile([128, TM, 128], dt.bfloat16, tag="selq")
                nc.vector.tensor_tensor(
                    out=selq[:, :T_rg, :],
                    in0=iota_row[:, None, :].to_broadcast([128, T_rg, 128]),
                    in1=st[i]["tc"][:, :T_rg].to_broadcast([128, T_rg, 128]),
                    op=OP.is_equal)
                st[i]["selT"], st[i]["selq"] = selT, selq

            def s1_expand(i):
                r, t0, T_rg, cells = spans[i]
                zb = zpa.tile([128, TM, HF], dt.float32, tag="zb")
                selT, xj = st[i]["selT"], st[i]["xj"]
                tb = [b for (b, tcnt) in cells for _ in range(tcnt)]
                done = 0
                while done < T_rg:
                    nsb = min(SB, T_rg - done)
                    psz = pz.tile([128, SB * HF], dt.float32, space="PSUM")
                    for j in range(nsb):
                        t = done + j
                        nc.tensor.matmul(
                            out=psz[:, HF * j:HF * (j + 1)],
                            lhsT=selT[:, 128 * t:128 * (t + 1)],
                            rhs=h_l_sb[:, tb[t] * HF:(tb[t] + 1) * HF],
                            start=True, stop=True)
                    nc.vector.tensor_tensor(
                        out=zb[:, done:done + nsb, :],
                        in0=psz[:, :nsb * HF].rearrange("p (t f) -> p t f",
                                                        f=HF),
                        in1=xj[:, done * HF:(done + nsb) * HF].rearrange(
                            "p (t f) -> p t f", f=HF),
                        op=OP.add)
                    done += nsb
                st[i]["zb"] = zb

            def s2_prelu(i):
                r, t0, T_rg, cells = spans[i]
                ub = zpb.tile([128, TM, HF], dt.bfloat16, tag="ub")
                nc.scalar.activation(out=ub[:, :T_rg, :],
                                     in_=st[i]["zb"][:, :T_rg, :],
                                     func=AF.Prelu, alpha=NEG_SLOPE)
                st[i]["ub"] = ub

            def s2_score(i):
                r, t0, T_rg, cells = spans[i]
                ub = st[i]["ub"]
                nc.vector.tensor_tensor(
                    out=ub[:, :T_rg, :], in0=ub[:, :T_rg, :],
                    in1=att_t[:, None, :].to_broadcast([128, T_rg, HF]),
                    op=OP.mult)
                sc = scp.tile([128, TM, H], dt.float32, tag="sc")
                nc.vector.tensor_reduce(
                    out=sc[:, :T_rg, :],
                    in_=ub[:, :T_rg, :].rearrange("p t (h f) -> p t h f", h=H),
                    axis=mybir.AxisListType.X, op=OP.add)
                st[i]["sc"] = sc

            def s3_exp(i):
                r, t0, T_rg, cells = spans[i]
                mx = mp.tile([128, TM, HF + H], dt.bfloat16, tag="mx")
                nc.scalar.activation(out=mx[:, :T_rg, HF:],
                                     in_=st[i]["sc"][:, :T_rg, :], func=AF.Exp)
                st[i]["mx"] = mx

            def s3_msg(i):
                r, t0, T_rg, cells = spans[i]
                mx, xj = st[i]["mx"], st[i]["xj"]
                nc.vector.tensor_tensor(
                    out=mx[:, :T_rg, :HF].rearrange("p t (h f) -> p t h f",
                                                    h=H),
                    in0=xj[:, :T_rg * HF].rearrange("p (t h f) -> p t h f",
                                                    h=H, f=F_OUT),
                    in1=mx[:, :T_rg, HF:].to_broadcast([128, T_rg, H, F_OUT]),
                    op=OP.mult)

            def s4_agg(i):
                r, t0, T_rg, cells = spans[i]
                mx, selq = st[i]["mx"], st[i]["selq"]
                t = 0
                for (b, tcnt) in cells:
                    psw = pw.tile([128, HF + H], dt.float32, space="PSUM")
                    for k in range(tcnt):
                        nc.tensor.matmul(out=psw[:], lhsT=selq[:, t + k, :],
                                         rhs=mx[:, t + k, :],
                                         start=(k == 0), stop=(k == tcnt - 1))
                    nc.vector.tensor_tensor(
                        out=acc_t[:, b * 68:b * 68 + 68],
                        in0=acc_t[:, b * 68:b * 68 + 68],
                        in1=psw[:], op=OP.add)
                    t += tcnt
                del st[i]

            hq = {r: list(range(CHUNK // 512)) for r in range(1, NCHUNK)}
            per_span_h = -(-(CHUNK // 512) // NSPAN)
            loads(0)
            if Nsp > 1:
                loads(1)
            gather(0)
            sels(0)
            for i in range(Nsp + 3):
                if 1 <= i < Nsp + 1:
                    s2_prelu(i - 1)
                if 2 <= i < Nsp + 2:
                    s3_exp(i - 2)
                if 3 <= i:
                    s4_agg(i - 3)
                if i + 2 < Nsp:
                    loads(i + 2)
                if i + 1 < Nsp:
                    gather(i + 1)
                    sels(i + 1)
                if i < Nsp:
                    r = spans[i][0]
                    if r + 1 < NCHUNK:
                        for _ in range(per_span_h):
                            if hq[r + 1]:
                                h_unit_table(r + 1, hq[r + 1].pop(0))
                if 2 <= i < Nsp + 2:
                    s3_msg(i - 2)
                if 1 <= i < Nsp + 1:
                    s2_score(i - 1)
                if i < Nsp:
                    s1_expand(i)

            # ---------------- finalize (7-block groups) -------------------
            accv = acc_t[:].rearrange("p (b c) -> p b c", c=HF + H)
            for b0 in range(0, NB, 7):
                nb7 = min(7, NB - b0)
                den = outp.tile([128, 7, H], dt.float32, tag="den")
                nc.vector.tensor_scalar(out=den[:, :nb7, :],
                                        in0=accv[:, b0:b0 + nb7, HF:],
                                        scalar1=1e-30, scalar2=None,
                                        op0=OP.max)
                rec = outp.tile([128, 7, H], dt.float32, tag="rec")
                nc.vector.reciprocal(out=rec[:, :nb7, :], in_=den[:, :nb7, :])
                ot = outp.tile([128, 7, HF], dt.float32, tag="ot")
                nc.vector.tensor_tensor(
                    out=ot[:, :nb7, :].rearrange("p b (h f) -> p b h f", h=H),
                    in0=accv[:, b0:b0 + nb7, :HF].rearrange(
                        "p b (h f) -> p b h f", h=H),
                    in1=rec[:, :nb7, :].to_broadcast([128, nb7, H, F_OUT]),
                    op=OP.mult)
                nc.vector.tensor_tensor(
                    out=ot[:, :nb7, :], in0=ot[:, :nb7, :],
                    in1=bias_t[:, None, :].to_broadcast([128, nb7, HF]),
                    op=OP.add)
                for j in range(nb7):
                    b = b0 + j
                    nc.sync.dma_start(out=out[128 * b:128 * (b + 1), :],
                                      in_=ot[:, j, :])

    nc.finalize()
    return nc


# ------------------------------------------------------------------- runner
_CACHE = {}


def _run(features, edge_index, weight_l, weight_r, att, bias, trace=False):
    from concourse.bass_utils import run_bass_kernel_spmd

    try:  # enable NTFF tracing under axon (missing antenv.axon_hooks shim)
        import antenv
        if "antenv.axon_hooks" not in sys.modules:
            from trn_agent_boot.trn_boot import _ntff_profile_via_ctypes
            hk = _ntff_profile_via_ctypes('/opt/axon/libaxon_pjrt.so')
            m = types.ModuleType("antenv.axon_hooks")
            m.get_axon_ntff_profile_hook = lambda: hk
            sys.modules["antenv.axon_hooks"] = m
            antenv.axon_hooks = m
    except Exception:
        pass

    features = np.asarray(features, dtype=np.float32)
    weight_l = np.asarray(weight_l, dtype=np.float32)
    weight_r = np.asarray(weight_r, dtype=np.float32)
    att = np.asarray(att, dtype=np.float32)
    bias = np.asarray(bias, dtype=np.float32)

    key, cores = prep(edge_index)
    if key not in _CACHE:
        _CACHE[key] = build(key)
    nc = _CACHE[key]

    featT = np.zeros((128, NPAD), np.float32)
    featT[:, :N] = np.ascontiguousarray(features.T)
    w_lrh = np.concatenate([weight_l, weight_r], axis=1)
    att_bh = np.tile(att.reshape(1, HF).astype(np.float32), (128, 1))
    bias_bh = np.tile(bias.reshape(1, HF), (128, 1))

    in_maps = []
    for c in range(NCORES):
        n0 = c * NLOC
        in_maps.append({
            "featT": featT,
            "featT_loc": np.ascontiguousarray(featT[:, n0:n0 + NLOCP]),
            "w_lr": w_lrh, "att_b": att_bh, "bias_b": bias_bh,
            **cores[c],
        })

    res = run_bass_kernel_spmd(nc, in_maps, core_ids=list(range(NCORES)),
                               trace=trace)
    full = np.empty((N, HF), np.float32)
    for c in range(NCORES):
        full[c * NLOC:(c + 1) * NLOC] = res.results[c]["out"][:NLOC]
    return full, res


def kernel(features, edge_index, weight_l, weight_r, att, bias):
    out, _ = _run(features, edge_index, weight_l, weight_r, att, bias)
    return out
